# revision 53
# baseline (speedup 1.0000x reference)
"""Trainium2 Bass kernel for ExactVisionAttention (block-diagonal attention).

Full inputs in, full outputs out. Sharding: segment-parallel over the 8
equal-length segments (attention is block-diagonal across segments), one
segment per NeuronCore. No collectives needed.

v3 dataflow per core (segment of 1024 tokens, HID=1280, 16 heads, D=80):
  Host prep (free): hidden pre-transposed + fp8-e4m3 hi/lo split; qkv weight
  COLUMNS REORDERED into head-group-major chunks (Q[g] K[g] V[g] for four
  groups of 4 heads) so each head group's attention starts while the QKV
  GEMM for later groups is still running; wproj hi/lo; cos/sin bf16.

  A: QKV via fp8 DoubleRow 3-term matmuls (exact to ~0.4%), emitted as
     per-(chunk, token-tile) units. Group 0's q,k chunks run up front
     behind a warm-up burst covering the input DMA ramp; the remaining 80
     units are pulled into the per-head schedule of phase B - concentrated
     LATE in each head's iteration so the QK matmuls issue early and pace
     the exp stream tightly - under per-group deadlines (q,k before the
     group's transposes, v before its AV). Evictions on ACT early / DVE
     late, RoPE on DVE (bf16). q/k/v sbuf tiles are per head-group so late
     A writes never false-conflict with early B reads.
  B: per head: q,k transposed on PE in 4 half-rounds through a shared psum
     ring; qT is quantized to single e4m3 (x32) on DVE, kT split hi/lo
     e4m3 (x32) on ACT+DVE. S^T = (k_hi,k_lo)^T (q8,q8) via one DoubleRow
     matmul per (kc, nn) at half the bf16 row cost; exp on ACT with the
     1/(SQ*SK) descale folded into the activation scale. AV is TOKEN-major:
     one matmul per (kc, qc) with P^T-slice stationary and v (+ones col)
     moving, out [128q, 81] in psum - 81-cycle matmuls put the head-dim
     waste in the cheap free axis and make the softmax sums per-partition
     scalars (col 80). PSUM allows only ONE open accumulation region per
     bank, so each query-block's 8-kc accumulation runs kc-innermost -
     which also makes AV dependency-free filler: head h's AV + normalize
     run one head behind the exp stream. Normalize = tiny DVE reciprocal +
     one scalar_tensor_tensor per psum half (SA prescale folded);
     transpose the normalized [128,80] tiles back to e-major on PE; hi/lo
     e4m3 split (ACT+DVE) into the DoubleRow-paired aT8 layout via
     SBUF->SBUF DMA.
  C: output projection, fp8 DoubleRow 3-term; leading 4-mt pair-outer
     group covers the last head's norm latency; descale folds into the
     f32 eviction before DMA out. (Note: matmul moving APs above ~1024
     elements fail neuronxcc ISA encoding despite simulating fine - QK
     stays as two 512-wide matmuls per key chunk.)

qkv_bias / proj_bias are zeros by construction (spec fill=zeros) and are
not applied. cu_seqlens is fixed equal segmentation and only validated.
"""

import os
import sys

for _p in ("/opt/trn_rl_repo", "/root/.axon_site", "/root/.axon_site/_ro/trn_rl_repo",
           "/root/.axon_site/_ro/pypackages"):
    if os.path.isdir(_p) and _p not in sys.path:
        sys.path.append(_p)

import numpy as np

S = 8192
HID = 1280
H = 16
D = 80
NSEG = 8
SEG = S // NSEG          # 1024 tokens per segment/core
MT = SEG // 128          # 8 token tiles per core
NP = 5                   # qkv contraction pairs (256 deep each)
SCALE = float(D) ** -0.5
SA = 64.0                # pow2 prescale for the normalized attention output
SQ = 32.0                # e4m3 prescale for q in the QK DoubleRow matmul
SK = 32.0                # e4m3 prescale for the k hi/lo split

GROUPS = [(0, 6), (6, 6), (12, 4)]
CHUNKS = []              # (kind, g, h0, nh, col_off) in reordered weight cols
_off = 0
for _g, (_h0, _nh) in enumerate(GROUPS):
    for _kind in range(3):
        CHUNKS.append((_kind, _g, _h0, _nh, _off))
        _off += _nh * D
NCH = len(CHUNKS)        # 9
TERMS = [(0, 0), (1, 0), (0, 1)]  # (sta lvl, mov lvl) fp8 3-term


def _group_of(h):
    for g, (h0, nh) in enumerate(GROUPS):
        if h0 <= h < h0 + nh:
            return g
    raise ValueError(h)


# A-units for groups 1,2 hosted inside the first 12 heads' schedules
A_BUDGET = [4, 4, 4, 4, 4, 4, 4, 4, 4, 4, 4, 4, 0, 0, 0, 0]

_CACHE = {}


def build_module(num_devices=8, repeat=1, inv_s=1.0 / (32.0 * 2048.0),
                 inv_p=1.0 / (64.0 * 2048.0), use_q8=True):
    import concourse.tile as tile
    from concourse import bacc, mybir
    from contextlib import ExitStack
    from collections import deque

    f32 = mybir.dt.float32
    bf16 = mybir.dt.bfloat16
    f8 = mybir.dt.float8e4
    Exp = mybir.ActivationFunctionType.Exp
    DR = mybir.MatmulPerfMode.DoubleRow
    Mult = mybir.AluOpType.mult
    Sub = mybir.AluOpType.subtract

    nc = bacc.Bacc("TRN2", target_bir_lowering=False, debug=False,
                   num_devices=num_devices)

    sta_in = nc.dram_tensor("sta8", [128, NP, 2, 2, SEG], f8,
                            kind="ExternalInput").ap()
    mov_in = nc.dram_tensor("mov8", [128, NP, 2, 2, 3 * HID], f8,
                            kind="ExternalInput").ap()
    cos_in = nc.dram_tensor("cosb", [128, MT, 40], bf16,
                            kind="ExternalInput").ap()
    sin_in = nc.dram_tensor("sinb", [128, MT, 40], bf16,
                            kind="ExternalInput").ap()
    wpj_in = nc.dram_tensor("wpj8", [128, NP, 2, 2, HID], f8,
                            kind="ExternalInput").ap()
    ident_in = nc.dram_tensor("identb", [128, 128], bf16,
                              kind="ExternalInput").ap()
    out_dram = nc.dram_tensor("out", [SEG, HID], f32, kind="ExternalOutput").ap()
    tag_dram = None
    if repeat > 1:
        tag_dram = nc.dram_tensor("rtag", [1, repeat], f32,
                                  kind="ExternalOutput").ap()

    with tile.TileContext(nc) as tc:
      for _rep in range(repeat):
        with ExitStack() as ctx:
            constp = ctx.enter_context(tc.tile_pool(name="const", bufs=1))
            projp = ctx.enter_context(tc.tile_pool(name="projp", bufs=1))
            stap = ctx.enter_context(tc.tile_pool(name="stap", bufs=1))
            movp = ctx.enter_context(tc.tile_pool(name="movp", bufs=2))
            qsp = ctx.enter_context(tc.tile_pool(name="qsp", bufs=6))
            rtp = ctx.enter_context(tc.tile_pool(name="rtp", bufs=4))
            qkv_ctx = ExitStack()
            qkvsb = qkv_ctx.enter_context(tc.tile_pool(name="qkvsb", bufs=1))

            # per-(group, token-tile) q/k/v tiles so interleaved A writes
            # don't alias B reads of earlier groups
            q_sb, k_sb, v_sb = {}, {}, {}
            for g, (h0, nh) in enumerate(GROUPS):
                for mt in range(MT):
                    q_sb[(g, mt)] = qkvsb.tile([128, nh, D], bf16,
                                               tag=f"q{g}_{mt}", name=f"q{g}_{mt}")
                    k_sb[(g, mt)] = qkvsb.tile([128, nh, D], bf16,
                                               tag=f"k{g}_{mt}", name=f"k{g}_{mt}")
                    v_sb[(g, mt)] = qkvsb.tile([128, nh, D + 1], bf16,
                                               tag=f"v{g}_{mt}", name=f"v{g}_{mt}")

            if tag_dram is not None:
                nc.sync.dma_start(tag_dram[:, _rep:_rep + 1],
                                  cos_in[0:1, 0:1, 0:1].rearrange("a b c -> a (b c)"))

            sta = stap.tile([128, NP, 2, 2, SEG], f8, tag="sta", name="sta")

            mov_tiles = {}

            def fetch_mov(ci):
                kind, g, h0, nh, off = CHUNKS[ci]
                w = nh * D
                m = movp.tile([128, NP, 2, 2, 480], f8, tag="mov",
                              name=f"mov{ci}")
                nc.sync.dma_start(m[:, :, :, :, 0:w],
                                  mov_in[:, :, :, :, off:off + w])
                mov_tiles[ci] = m

            # DMA order: pair 0 of chunk 0 + sta pair 0 first so the PE can
            # start early; the rest streams behind the warmup matmuls.
            _, _, _, nh0, off0 = CHUNKS[0]
            w0 = nh0 * D
            m0 = movp.tile([128, NP, 2, 2, 480], f8, tag="mov", name="mov0")
            nc.sync.dma_start(m0[:, 0, :, :, 0:w0],
                              mov_in[:, 0, :, :, off0:off0 + w0])
            nc.sync.dma_start(sta[:, 0, 0], sta_in[:, 0, 0])
            nc.sync.dma_start(sta[:, 0, 1], sta_in[:, 0, 1])
            for p in range(1, NP):
                nc.sync.dma_start(m0[:, p, :, :, 0:w0],
                                  mov_in[:, p, :, :, off0:off0 + w0])
                nc.sync.dma_start(sta[:, p], sta_in[:, p])
            mov_tiles[0] = m0
            fetch_mov(1)

            cosb = constp.tile([128, MT, 40], bf16, tag="cosb", name="cosb")
            sinb = constp.tile([128, MT, 40], bf16, tag="sinb", name="sinb")
            ident = constp.tile([128, 128], bf16, tag="ident", name="ident")
            nc.sync.dma_start(cosb[:], cos_in[:])
            nc.sync.dma_start(sinb[:], sin_in[:])
            nc.sync.dma_start(ident[:], ident_in[:])
            # warm-up operand first in the DVE queue so PE warm-up matmuls
            # start immediately; the v ones-columns follow
            warm = qsp.tile([128, 6, D], bf16, tag="qs", name="warm")
            nc.vector.memset(warm[:], 0.0)
            for g in range(len(GROUPS)):
                for mt in range(MT):
                    nc.vector.memset(v_sb[(g, mt)][:, :, D:D + 1], 1.0)

            def emit_rope(dst, src, nh, mt):
                # dst = src*cos + rotate_half(src)*sin, all bf16 on DVE
                src3 = src[:, 0:nh, :]
                src4 = src3.rearrange("p h (two d) -> p h two d", two=2)
                cos_bc4 = (cosb[:, mt].unsqueeze(1).unsqueeze(2)
                           .broadcast_to([128, nh, 2, 40]))
                sin_bc3 = (sinb[:, mt].unsqueeze(1)
                           .broadcast_to([128, nh, 40]))
                t = rtp.tile([128, 6, D], bf16, tag="t", name="t")
                t4 = t[:, 0:nh, :].rearrange("p h (two d) -> p h two d",
                                             two=2)
                nc.vector.tensor_mul(t4, src4, cos_bc4)
                m1 = rtp.tile([128, 6, 40], bf16, tag="m1", name="m1")
                nc.vector.tensor_mul(m1[:, 0:nh, :], src3[:, :, 40:80],
                                     sin_bc3)
                m2 = rtp.tile([128, 6, 40], bf16, tag="m2", name="m2")
                nc.vector.tensor_mul(m2[:, 0:nh, :], src3[:, :, 0:40],
                                     sin_bc3)
                nc.vector.tensor_sub(dst[:, :, 0:40], t[:, 0:nh, 0:40],
                                     m1[:, 0:nh, :])
                nc.vector.tensor_add(dst[:, :, 40:80], m2[:, 0:nh, :],
                                     t[:, 0:nh, 40:80])

            def make_A_emitter(psA):
                def emit_A_unit(ci, mt, act_evict=True):
                    kind, g, h0, nh, off = CHUNKS[ci]
                    w = nh * D
                    # prefetch the chunk 2 slots ahead only once ALL of this
                    # chunk's units are emitted: the mov ring slot it reuses
                    # must already have every consumer registered
                    if mt == MT - 1 and ci + 2 < NCH and (ci + 2) not in mov_tiles:
                        fetch_mov(ci + 2)
                    mov = mov_tiles[ci]
                    ps = psA.tile([128, 512], f32, tag="m", name=f"a{ci}_{mt}")
                    pw = ps[:, 0:w]
                    for pi in range(NP):
                        for ti, (sl, ml) in enumerate(TERMS):
                            nc.tensor.matmul(
                                pw,
                                sta[:, pi, sl, :, mt * 128:(mt + 1) * 128],
                                mov[:, pi, ml, :, 0:w],
                                start=(pi == 0 and ti == 0),
                                stop=(pi == NP - 1 and ti == 2),
                                perf_mode=DR)
                    ps3 = pw.rearrange("p (h d) -> p h d", h=nh)
                    if kind == 2:
                        nc.vector.tensor_scalar_mul(
                            v_sb[(g, mt)][:, :, 0:D], ps3, inv_s)
                    else:
                        qs = qsp.tile([128, 6, D], bf16, tag="qs", name="qs")
                        if act_evict:
                            nc.scalar.mul(qs[:, 0:nh, :], ps3, inv_s)
                        else:
                            nc.vector.tensor_scalar_mul(qs[:, 0:nh, :], ps3,
                                                        inv_s)
                        dst = q_sb[(g, mt)] if kind == 0 else k_sb[(g, mt)]
                        emit_rope(dst[:, :, :], qs, nh, mt)
                return emit_A_unit

            # ------------- B pools + A interleave (A group 0 first) -------
            aT8 = [[projp.tile([128, 2, SEG], f8, tag=f"aT{lv}_{P}",
                               name=f"aT{lv}_{P}") for P in range(NP)]
                   for lv in range(2)]

            with ExitStack() as bctx:
                sbB = bctx.enter_context(tc.tile_pool(name="sbB", bufs=2))
                psS = bctx.enter_context(
                    tc.tile_pool(name="psS", bufs=2, space="PSUM"))
                psM = bctx.enter_context(
                    tc.tile_pool(name="psM", bufs=2, space="PSUM"))
                psO = bctx.enter_context(
                    tc.tile_pool(name="psO", bufs=2, space="PSUM"))

                emit_A = make_A_emitter(psM)
                a_queue = deque((ci, mt) for ci in range(NCH)
                                for mt in range(MT))

                wps = psM.tile([128, 512], f32, tag="m", name="warm_ps")
                wflat = warm[:].rearrange("p h d -> p (h d)")
                for _ in range(14):
                    nc.tensor.matmul(wps[:, 0:480], wflat[:, 0:128], wflat[:],
                                     start=True, stop=True)

                def emit_A_paired(ci, mts):
                    # pair-outer variant for the DMA-paced first chunks:
                    # two token-tiles' psums accumulate across weight pairs
                    # in arrival order, so the PE tracks the input DMA
                    kind, g, h0, nh, off = CHUNKS[ci]
                    w = nh * D
                    if mts[0] == 0 and ci + 3 < NCH and (ci + 3) not in mov_tiles:
                        fetch_mov(ci + 3)
                    mov = mov_tiles[ci]
                    pss = [psM.tile([128, 480], f32, tag="m",
                                    name=f"a{ci}_{mt}") for mt in mts]
                    for pi in range(NP):
                        for ti, (sl, ml) in enumerate(TERMS):
                            for i, mt in enumerate(mts):
                                nc.tensor.matmul(
                                    pss[i][:, 0:w],
                                    sta[:, pi, sl, :, mt * 128:(mt + 1) * 128],
                                    mov[:, pi, ml, :, 0:w],
                                    start=(pi == 0 and ti == 0),
                                    stop=(pi == NP - 1 and ti == 2),
                                    perf_mode=DR)
                    for i, mt in enumerate(mts):
                        ps3 = pss[i][:, 0:w].rearrange("p (h d) -> p h d",
                                                       h=nh)
                        qs = qsp.tile([128, 6, D], bf16, tag="qs", name="qs")
                        nc.scalar.mul(qs[:, 0:nh, :], ps3, inv_s)
                        dst = q_sb[(g, mt)] if kind == 0 else k_sb[(g, mt)]
                        emit_rope(dst[:, :, :], qs, nh, mt)

                act_late = [False]

                def pull_A(n=1):
                    for _ in range(n):
                        if a_queue:
                            emit_A(*a_queue.popleft(),
                                   act_evict=not act_late[0])

                def drain_A_to(last_ci):
                    while a_queue and a_queue[0][0] <= last_ci:
                        emit_A(*a_queue.popleft(),
                               act_evict=not act_late[0])

                hs = [{"p": [None] * MT, "o": [None, None], "T": None,
                       "on": None} for _ in range(H)]

                def alloc_T(h):
                    if use_q8:
                        q8T = sbB.tile([80, SEG], f8, tag="q8T", bufs=3,
                                       name=f"q8T{h}")
                        kT8 = sbB.tile([80, 2, SEG], f8, tag="kT8", bufs=3,
                                       name=f"kT8{h}")
                        hs[h]["T"] = (q8T, kT8)
                    else:
                        qT = sbB.tile([80, SEG], bf16, tag="q8T", bufs=3,
                                      name=f"qT{h}")
                        kT = sbB.tile([80, SEG], bf16, tag="kT8", bufs=3,
                                      name=f"kT{h}")
                        hs[h]["T"] = (qT, kT)

                def emit_T(h, which, half):
                    # transpose 4 token-tiles of q or k into the psum ring,
                    # then evict: q -> single e4m3 (x SQ) ; k -> hi/lo e4m3
                    g = _group_of(h)
                    h0 = GROUPS[g][0]
                    tq = psM.tile([80, 512], bf16, tag="m",
                                  name=f"tq{h}_{which}{half}")
                    src = q_sb if which == 0 else k_sb
                    with nc.allow_low_precision(reason="transpose is exact"):
                        for i in range(4):
                            mt = half * 4 + i
                            nc.tensor.transpose(tq[:, i * 128:(i + 1) * 128],
                                                src[(g, mt)][:, h - h0, :],
                                                ident[:])
                    cs = slice(half * 512, (half + 1) * 512)
                    if use_q8:
                        with nc.allow_low_precision(
                                reason="fp8 attention, error budget checked"):
                            if which == 0:
                                nc.vector.tensor_scalar_mul(
                                    hs[h]["T"][0][:, cs], tq[:], SQ)
                            else:
                                kT8 = hs[h]["T"][1]
                                nc.vector.tensor_scalar_mul(
                                    kT8[:, 0, cs], tq[:], SK)
                                nc.vector.scalar_tensor_tensor(
                                    kT8[:, 1, cs], tq[:], SK, kT8[:, 0, cs],
                                    Mult, Sub)
                    else:
                        nc.vector.tensor_copy(hs[h]["T"][which][:, cs], tq[:])

                def emit_qk(h, kc):
                    s_ps = psS.tile([128, SEG], f32, tag="s", name="s_ps")
                    if use_q8:
                        q8T, kT8 = hs[h]["T"]
                        for nn in range(2):
                            q_mov = (q8T[:, nn * 512:(nn + 1) * 512]
                                     .unsqueeze(1).broadcast_to([80, 2, 512]))
                            nc.tensor.matmul(
                                s_ps[:, nn * 512:(nn + 1) * 512],
                                kT8[:, :, kc * 128:(kc + 1) * 128],
                                q_mov, start=True, stop=True, perf_mode=DR)
                        escale = SCALE / (SQ * SK)
                    else:
                        qT, kT = hs[h]["T"]
                        for nn in range(2):
                            nc.tensor.matmul(
                                s_ps[:, nn * 512:(nn + 1) * 512],
                                kT[:, kc * 128:(kc + 1) * 128],
                                qT[:, nn * 512:(nn + 1) * 512],
                                start=True, stop=True)
                        escale = SCALE
                    p_sb = sbB.tile([128, SEG], bf16, tag="p", name="p_sb",
                                    bufs=14)
                    nc.scalar.activation(p_sb[:], s_ps[:], Exp, scale=escale)
                    hs[h]["p"][kc] = p_sb

                def emit_av(h, qc):
                    # one query-block's full 1024-key accumulation, emitted
                    # back-to-back: psum allows only ONE open accumulation
                    # region per bank, so the kc loop must be innermost.
                    # All of head h's p tiles exist by now (AV runs one head
                    # behind the exp stream and is dependency-free filler).
                    if hs[h]["o"][0] is None:
                        hs[h]["o"] = [psO.tile([128, 4, D + 1], f32,
                                               tag="oT", name=f"o{h}_{i}")
                                      for i in range(2)]
                    g = _group_of(h)
                    h0 = GROUPS[g][0]
                    ot = hs[h]["o"][qc // 4]
                    j = qc % 4
                    for kc in range(MT):
                        nc.tensor.matmul(
                            ot[:, j, :],
                            hs[h]["p"][kc][:, qc * 128:(qc + 1) * 128],
                            v_sb[(g, kc)][:, h - h0, :],
                            start=(kc == 0), stop=(kc == MT - 1))

                def emit_norm_a(h, halves=(0, 1)):
                    # normalize+prescale the token-major psum halves into
                    # bf16 (frees the o-psum ring for the next head)
                    if hs[h]["on"] is None:
                        hs[h]["on"] = sbB.tile([128, MT, D], bf16, tag="on",
                                               name=f"on{h}", bufs=2)
                    on = hs[h]["on"]
                    with nc.allow_low_precision(
                            reason="softmax sums ~1e3, bf16 recip err 0.4%"):
                        for half in halves:
                            ot = hs[h]["o"][half]
                            rb = sbB.tile([128, 4], bf16, tag="rb", bufs=4,
                                          name="rb")
                            nc.vector.reciprocal(
                                rb[:].unsqueeze(2), ot[:, :, D:D + 1])
                            nc.vector.scalar_tensor_tensor(
                                on[:, half * 4:(half + 1) * 4, :],
                                ot[:, :, 0:D], SA,
                                rb[:].unsqueeze(2).broadcast_to([128, 4, D]),
                                Mult, Mult)

                def emit_norm_b(h):
                    # transpose back to e-major; hi/lo e4m3 split into the
                    # DoubleRow-paired aT8 layout via SBUF->SBUF DMA
                    on = hs[h]["on"]
                    otp = psM.tile([D, SEG], bf16, tag="m", name=f"otp{h}")
                    with nc.allow_low_precision(reason="transpose is exact"):
                        for qc in range(MT):
                            nc.tensor.transpose(
                                otp[:, qc * 128:(qc + 1) * 128],
                                on[:, qc, :], ident[:])
                    with nc.allow_low_precision(
                            reason="hi/lo e4m3 pair carries ~8 mantissa bits"):
                        hi = sbB.tile([D, SEG], f8, tag="hi", name="hi", bufs=1)
                        if h >= 9:
                            # tail heads are exp(ACT)-bound; DVE has slack
                            nc.vector.tensor_copy(hi[:], otp[:])
                        else:
                            nc.scalar.copy(hi[:], otp[:])
                        lo = sbB.tile([D, SEG], f8, tag="lo", name="lo", bufs=1)
                        nc.vector.tensor_sub(lo[:], otp[:], hi[:])
                    e0 = h * D
                    pieces = []
                    while e0 < (h + 1) * D:
                        ln = min(128 - e0 % 128, (h + 1) * D - e0)
                        pieces.append((e0, ln))
                        e0 += ln
                    for lv, src in ((0, hi), (1, lo)):
                        for (es, ln) in pieces:
                            kc, r0 = es // 128, es % 128
                            off = es - h * D
                            nc.sync.dma_start(
                                aT8[lv][kc // 2][r0:r0 + ln, kc % 2, :],
                                src[off:off + ln, :])

                # ---- head loop ----
                # q0,k0 chunks first; transposes + first QKs of heads 0/1
                # before the v0 chunk so the exp stream starts early
                drain_A_to(1)
                alloc_T(0)
                for w in range(2):
                    for half in range(2):
                        emit_T(0, w, half)
                emit_qk(0, 0)
                emit_qk(0, 1)
                wpj = projp.tile([128, NP, 2, 2, HID], f8, tag="wpj",
                                 name="wpj")
                nc.sync.dma_start(wpj[:], wpj_in[:])
                pend = None
                for h in range(H):
                    act_late[0] = h >= 8
                    budget = A_BUDGET[h]
                    av = h - 1  # AV runs one head behind the exp stream
                    if av >= 0:
                        # v chunk of av's group must precede its AV blocks
                        drain_A_to(3 * _group_of(av) + 2)
                    emit_qk(h, 2)
                    if av >= 0:
                        emit_av(av, 0)
                        emit_av(av, 1)
                    # T rounds before the A pulls: their DVE evicts then
                    # precede the pulls' rope chains in the DVE queue
                    if h + 1 < H:
                        # q,k chunks of h+1's group must precede its transposes
                        drain_A_to(3 * _group_of(h + 1) + 1)
                        alloc_T(h + 1)
                        emit_T(h + 1, 0, 0)
                        emit_T(h + 1, 0, 1)
                    emit_qk(h, 3)
                    if av >= 0:
                        emit_av(av, 2)
                        emit_av(av, 3)
                    if budget > 0:
                        pull_A()
                    if h + 1 < H:
                        emit_T(h + 1, 1, 0)
                        emit_T(h + 1, 1, 1)
                    if budget > 1:
                        pull_A()
                    emit_qk(h, 4)
                    if av >= 0:
                        emit_av(av, 4)
                        emit_av(av, 5)
                    if budget > 2:
                        pull_A()
                    emit_qk(h, 5)
                    if av >= 0:
                        emit_av(av, 6)
                        emit_av(av, 7)
                        emit_norm_a(av)
                    if budget > 3:
                        pull_A()
                    emit_qk(h, 6)
                    if av >= 0:
                        emit_norm_b(av)
                    if budget > 4:
                        pull_A()
                    emit_qk(h, 7)
                    if h + 1 < H:
                        emit_qk(h + 1, 0)
                        emit_qk(h + 1, 1)
                # drain: last head's AV + norm; the first normalize half
                # only needs the first four AV blocks, so it hides under
                # the remaining AV matmuls
                for qc in range(4):
                    emit_av(H - 1, qc)
                emit_norm_a(H - 1, halves=(0,))
                for qc in range(4, MT):
                    emit_av(H - 1, qc)
                emit_norm_a(H - 1, halves=(1,))
                emit_norm_b(H - 1)
                assert not a_queue, "unemitted A units"

            qkv_ctx.close()  # q/k/v dead after attention; free for phase C

            # ---------------- Phase C: output projection ----------------
            with ExitStack() as cctx:
                osbp = cctx.enter_context(tc.tile_pool(name="osbp", bufs=1))
                psC = cctx.enter_context(
                    tc.tile_pool(name="psC", bufs=1, space="PSUM"))
                NTC3 = list(enumerate([(0, 512), (512, 512), (1024, 256)]))
                terms = [(0, 0), (1, 0), (0, 1)]  # (aT lvl, w lvl)
                bank_ctr = [0]

                def emit_proj(mts, ntc, ots):
                    pss = {}
                    for i, mt in enumerate(mts):
                        for j, (n0, nw) in ntc:
                            pss[(i, j)] = psC.tile(
                                [128, nw], f32,
                                tag=f"b{bank_ctr[0] % 8}", name="pc", bufs=1)
                            bank_ctr[0] += 1
                    for pi in range(NP):
                        for ti, (al, wl) in enumerate(terms):
                            for i, mt in enumerate(mts):
                                for j, (n0, nw) in ntc:
                                    nc.tensor.matmul(
                                        pss[(i, j)][:],
                                        aT8[al][pi][:, :,
                                                    mt * 128:(mt + 1) * 128],
                                        wpj[:, pi, wl, :, n0:n0 + nw],
                                        start=(pi == 0 and ti == 0),
                                        stop=(pi == NP - 1 and ti == 2),
                                        perf_mode=DR)
                    c0 = min(n0 for _, (n0, _) in ntc)
                    c1 = max(n0 + nw for _, (n0, nw) in ntc)
                    for i, mt in enumerate(mts):
                        for j, (n0, nw) in ntc:
                            dst = ots[i][:, n0:n0 + nw]
                            if j == 1:
                                nc.scalar.mul(dst, pss[(i, j)][:], inv_p)
                            else:
                                nc.vector.tensor_scalar_mul(
                                    dst, pss[(i, j)][:], inv_p)
                        nc.sync.dma_start(
                            out_dram[mt * 128:(mt + 1) * 128, c0:c1],
                            ots[i][:, c0:c1])

                ots = [osbp.tile([128, HID], f32, tag=f"osb{i}",
                                 name=f"osb{i}", bufs=1)
                       for i in (0, 1, 3, 4)]
                emit_proj([0, 1, 2, 3], NTC3[:2], ots)
                emit_proj([0, 1, 2, 3], NTC3[2:], ots)
                for mt in range(4, MT - 1):
                    ot = osbp.tile([128, HID], f32, tag=f"osb{mt % 2}",
                                   name="osb", bufs=1)
                    emit_proj([mt], NTC3, [ot])
                ot = osbp.tile([128, HID], f32, tag="osb2", name="osb2",
                               bufs=1)
                for j in range(3):
                    emit_proj([MT - 1], NTC3[j:j + 1], [ot])

    nc.compile()
    return nc


def _pow2scale(x):
    m = float(np.abs(x).max())
    return float(2.0 ** np.floor(np.log2(256.0 / m))) if m > 0 else 1.0


def _hilo8(x):
    import ml_dtypes
    hi = x.astype(ml_dtypes.float8_e4m3fn)
    lo = (x - hi.astype(np.float32)).astype(ml_dtypes.float8_e4m3fn)
    return hi, lo


def _col_order():
    cols = []
    for kind, g, h0, nh, off in CHUNKS:
        for h in range(h0, h0 + nh):
            for d in range(D):
                cols.append(kind * HID + h * D + d)
    return np.array(cols, dtype=np.int64)


def kernel(hidden_states, cos, sin, qkv_kernel, qkv_bias, proj_kernel,
           proj_bias, cu_seqlens):
    import ml_dtypes
    from concourse import bass_utils

    hidden_states = np.ascontiguousarray(hidden_states, dtype=np.float32)
    wqkv = np.ascontiguousarray(
        np.asarray(qkv_kernel, dtype=np.float32).reshape(HID, 3 * H * D))
    wproj = np.ascontiguousarray(proj_kernel, dtype=np.float32)

    assert not np.any(np.asarray(qkv_bias)), "nonzero qkv_bias unsupported"
    assert not np.any(np.asarray(proj_bias)), "nonzero proj_bias unsupported"
    expected_cu = np.arange(NSEG + 1, dtype=np.int64) * SEG
    assert np.array_equal(np.asarray(cu_seqlens, dtype=np.int64), expected_cu), \
        "kernel specialized for equal 1024-token segments"

    sh = _pow2scale(hidden_states)
    sw = _pow2scale(wqkv)
    swp = _pow2scale(wproj)
    inv_s = 1.0 / (sh * sw)
    inv_p = 1.0 / (SA * swp)

    key = ("nc3", NSEG, inv_s, inv_p)
    if key not in _CACHE:
        _CACHE[key] = build_module(num_devices=NSEG, inv_s=inv_s, inv_p=inv_p)
    nc = _CACHE[key]

    # weights: reordered columns, packed per (pair, hi/lo, slot) fp8 layout
    wqkv_r = np.ascontiguousarray(wqkv[:, _col_order()])
    wh, wl = _hilo8(wqkv_r * sw)
    mov8 = np.ascontiguousarray(np.stack(
        [w.reshape(NP, 2, 128, 3 * HID).transpose(2, 0, 1, 3)
         for w in (wh, wl)], axis=2))       # [128, NP, 2lvl, 2slot, 3840]
    ph, pl = _hilo8(wproj * swp)
    wpj8 = np.ascontiguousarray(np.stack(
        [w.reshape(NP, 2, 128, HID).transpose(2, 0, 1, 3)
         for w in (ph, pl)], axis=2))       # [128, NP, 2lvl, 2slot, HID]
    identb = np.eye(128, dtype=ml_dtypes.bfloat16)

    in_maps = []
    for c in range(NSEG):
        rows = slice(c * SEG, (c + 1) * SEG)
        hidT = np.ascontiguousarray(hidden_states[rows].T) * sh  # [1280,1024]
        hh, hl = _hilo8(hidT)
        sta8 = np.stack(
            [x.reshape(NP, 2, 128, SEG).transpose(2, 0, 1, 3)
             for x in (hh, hl)], axis=2)       # [128, NP, 2lvl, 2slot, SEG]
        sta8 = np.ascontiguousarray(sta8)
        cosb = np.ascontiguousarray(
            np.asarray(cos[rows, 0:40]).reshape(MT, 128, 40).transpose(1, 0, 2)
            .astype(ml_dtypes.bfloat16))
        sinb = np.ascontiguousarray(
            np.asarray(sin[rows, 0:40]).reshape(MT, 128, 40).transpose(1, 0, 2)
            .astype(ml_dtypes.bfloat16))
        in_maps.append({
            "sta8": sta8,
            "mov8": mov8,
            "cosb": cosb,
            "sinb": sinb,
            "wpj8": wpj8,
            "identb": identb,
        })

    res = bass_utils.run_bass_kernel_spmd(nc, in_maps,
                                          core_ids=list(range(NSEG)))
    out = np.concatenate([res.results[c]["out"] for c in range(NSEG)], axis=0)
    return out.astype(np.float32)
